# revision 1
# baseline (speedup 1.0000x reference)
"""Causal temporal attention kernel for 8 Trainium2 NeuronCores.

Reference computation (per batch b):
    qkv = x @ w_qkv + b_qkv ; split into q,k,v heads [H=16, Dh=64]
    q += pos_bias ; S = q k^T * Dh^-0.5 ; causal softmax ; out = S v
    y = concat_heads(out) @ w_out + b_out

Sharding: batch 2-way x head-group 4-way -> 8 cores. Core c = b*4 + g
computes heads 4g..4g+3 of batch b and returns the partial
y_part = concat(out_heads) @ w_out[rows of its heads]  ([T, DIM]).
Host sums the 4 partials per batch and adds b_out.

On-core layout is fully transposed so no PE transposes are needed:
    QT/KT pair tiles [128(2 heads x 64d), T], V as AV-ready lhsT chunks
    [128k, 65] (65th column = ones so the AV matmul also produces the
    softmax denominator), S^T tiles [128k, 512q] -> exp on ACT ->
    PT [128k, 512q] -> AV accumulates outT [65, 512q] in PSUM.
    Normalization r = 1/sums uses reciprocal_approx_fast and a DRAM
    round-trip for the partition broadcast (DMA can broadcast from
    DRAM; compute engines cannot cross partitions). All matmuls use
    float32r (full-rate fp32, ~1.5e-4 relative error). The two heads
    of a pair occupy partition rows 0-63 / 64-127, so their K=64
    S^T matmuls land in distinct PE row-groups and run concurrently.
"""

import sys

sys.path.insert(0, "/opt/trn_rl_repo")

from contextlib import ExitStack

import numpy as np

import concourse.bacc as bacc
import concourse.tile as tile
from concourse import mybir
from concourse.bass_utils import run_bass_kernel_spmd

F32 = mybir.dt.float32
F32R = mybir.dt.float32r
EXP = mybir.ActivationFunctionType.Exp

B, T, DIM = 2, 2048, 1024
HEADS, DH = 16, 64
HPC = 4              # heads per core
NCORES = 8
SCALE = DH ** -0.5
QT_TILES = T // 512  # 4 q-tiles of 512
KCH = T // 128       # 16 k-chunks of 128
VSTRIDE = KCH * 65   # per-head stride in v_sb


def _build_nc():
    nc = bacc.Bacc("TRN2", target_bir_lowering=False, debug=False,
                   num_devices=NCORES)
    xt_d = nc.dram_tensor("xt", [DIM, T], F32, kind="ExternalInput").ap()
    wqk_d = nc.dram_tensor("wqk", [DIM, 512], F32, kind="ExternalInput").ap()
    wv_d = nc.dram_tensor("wv", [DIM, HPC * DH], F32, kind="ExternalInput").ap()
    qb_d = nc.dram_tensor("qbias", [128, 2], F32, kind="ExternalInput").ap()
    kb_d = nc.dram_tensor("kbias", [128, 2], F32, kind="ExternalInput").ap()
    bvb_d = nc.dram_tensor("bvb", [128, HPC * DH], F32, kind="ExternalInput").ap()
    wout_d = nc.dram_tensor("wout", [2, 128, DIM], F32, kind="ExternalInput").ap()
    mask_d = nc.dram_tensor("masks", [4, 128, 512], F32, kind="ExternalInput").ap()
    id_d = nc.dram_tensor("ident", [128, 128], F32, kind="ExternalInput").ap()
    y_d = nc.dram_tensor("y", [T, DIM], F32, kind="ExternalOutput").ap()
    rb_d = nc.dram_tensor("rbscratch", [2 * QT_TILES * 2, 512], F32).ap()
    rb2_d = nc.dram_tensor("rbscratch2", [2 * QT_TILES * 2, 512], F32).ap()

    with tile.TileContext(nc) as tc, ExitStack() as ctx:
        res = ctx.enter_context(tc.tile_pool(name="res", bufs=1))
        small = ctx.enter_context(tc.tile_pool(name="small", bufs=8))

        # ---- PE warm-up burst: dense dependency-free matmuls while the
        # input DMAs stream in, so the HAM clock gate releases early.
        ones_f = small.tile([128, 512], F32, tag="ones_f")
        nc.any.memset(ones_f[:], 1.0)
        warm = res.tile([1, 512], F32R, tag="warm")
        nc.vector.tensor_copy(warm[:], ones_f[0:1, :])
        ones64 = res.tile([1, 64], F32R, tag="ones64")
        nc.vector.tensor_copy(ones64[:], ones_f[0:1, 0:64])
        with tc.tile_pool(name="psW", bufs=2, space="PSUM") as psW:
            for i in range(24):
                wp = psW.tile([64, 512], F32, tag="warm_ps", name=f"warm{i}")
                nc.tensor.matmul(wp[:], ones64[:], warm[:], start=True, stop=True)

        # ---- resident tiles ----
        wout_t = []
        for p in range(2):
            w = res.tile([128, DIM], F32R, tag=f"wout{p}", name=f"wout{p}")
            nc.scalar.dma_start(w[:], wout_d[p].bitcast(F32R))
            wout_t.append(w)
        mask_t = []
        for j in range(4):
            m = res.tile([128, 512], F32R, tag=f"mask{j}", name=f"mask{j}")
            nc.scalar.dma_start(m[:], mask_d[j].bitcast(F32R))
            mask_t.append(m)
        ident = res.tile([128, 128], F32R, tag="ident")
        nc.scalar.dma_start(ident[:], id_d[:, :].bitcast(F32R))
        qb = res.tile([128, 2], F32, tag="qb")
        nc.scalar.dma_start(qb[:], qb_d[:, :])
        kb = res.tile([128, 2], F32, tag="kb")
        nc.scalar.dma_start(kb[:], kb_d[:, :])
        bvb = res.tile([128, HPC * DH], F32, tag="bvb")
        nc.scalar.dma_start(bvb[:], bvb_d[:, :])

        qt_sb, kt_sb, outT = [], [], []
        for p in range(2):
            qt_sb.append(res.tile([128, T], F32R, tag=f"qt{p}", name=f"qt{p}"))
            kt_sb.append(res.tile([128, T], F32R, tag=f"kt{p}", name=f"kt{p}"))
            outT.append(res.tile([128, T], F32R, tag=f"outT{p}", name=f"outT{p}"))
        v_sb = res.tile([128, HPC * VSTRIDE], F32R, tag="v_sb")

        # ---- phase A: qkv projection (pools close -> SBUF/PSUM reused) ----
        with tc.tile_pool(name="phA", bufs=1) as phA:
            xt, wqk_t, wv_t = [], [], []
            for c in range(8):
                w = phA.tile([128, 512], F32R, tag=f"wqk{c}", name=f"wqk{c}")
                nc.sync.dma_start(w[:], wqk_d[c * 128:(c + 1) * 128, :].bitcast(F32R))
                wqk_t.append(w)
                w = phA.tile([128, HPC * DH], F32R, tag=f"wv{c}", name=f"wv{c}")
                nc.sync.dma_start(w[:], wv_d[c * 128:(c + 1) * 128, :].bitcast(F32R))
                wv_t.append(w)
                t_ = phA.tile([128, T], F32R, tag=f"xt{c}", name=f"xt{c}")
                nc.sync.dma_start(t_[:], xt_d[c * 128:(c + 1) * 128, :].bitcast(F32R))
                xt.append(t_)

            # QT / KT: 16 accumulation groups in 4 blocks of 4 banks,
            # contraction (c) outer inside a block so the PE can start on
            # each xt chunk the moment its DMA lands.
            groups = [(qk, p, tt) for p in range(2) for qk in range(2)
                      for tt in range(QT_TILES)]
            with tc.tile_pool(name="psQ", bufs=4, space="PSUM") as psQ:
                for blk in range(0, 16, 4):
                    ps_blk = []
                    for gi in range(4):
                        ps = psQ.tile([128, 512], F32, tag="qkps",
                                      name=f"qkps{blk + gi}")
                        ps_blk.append(ps)
                    for c in range(8):
                        for gi in range(4):
                            qk, p, tt = groups[blk + gi]
                            nc.tensor.matmul(
                                ps_blk[gi][:],
                                wqk_t[c][:, (qk * 2 + p) * 128:(qk * 2 + p + 1) * 128],
                                xt[c][:, tt * 512:(tt + 1) * 512],
                                start=(c == 0), stop=(c == 7))
                    for gi in range(4):
                        qk, p, tt = groups[blk + gi]
                        dst = (qt_sb if qk == 0 else kt_sb)[p]
                        bias = (qb if qk == 0 else kb)[:, p:p + 1]
                        nc.vector.tensor_add(
                            dst[:, tt * 512:(tt + 1) * 512], ps_blk[gi][:],
                            bias.to_broadcast((128, 512)))

            # V: out[128t, 256d] accum over 8 c-chunks; scatter into v_sb
            with tc.tile_pool(name="psV", bufs=4, space="PSUM") as psV:
                for mb in range(0, KCH, 4):
                    ps_blk = []
                    for mi in range(4):
                        ps = psV.tile([128, HPC * DH], F32, tag="vps",
                                      name=f"vps{mb + mi}")
                        ps_blk.append(ps)
                    for c in range(8):
                        for mi in range(4):
                            m = mb + mi
                            nc.tensor.matmul(ps_blk[mi][:],
                                             xt[c][:, m * 128:(m + 1) * 128],
                                             wv_t[c][:], start=(c == 0),
                                             stop=(c == 7))
                    for mi in range(4):
                        m = mb + mi
                        for h in range(HPC):
                            off = h * VSTRIDE + m * 65
                            nc.vector.tensor_add(v_sb[:, off:off + 64],
                                                 ps_blk[mi][:, h * DH:(h + 1) * DH],
                                                 bvb[:, h * DH:(h + 1) * DH])
                            nc.vector.tensor_copy(v_sb[:, off + 64:off + 65],
                                                  ones_f[:, 0:1])

        # ---- phases B/C: attention + output projection ----
        with tc.tile_pool(name="phB", bufs=6) as phB, \
             tc.tile_pool(name="ptp", bufs=12) as ptp, \
             tc.tile_pool(name="rbp", bufs=6) as rbp, \
             tc.tile_pool(name="psB", bufs=2, space="PSUM") as psB, \
             tc.tile_pool(name="psO", bufs=2, space="PSUM") as psO, \
             tc.tile_pool(name="psY", bufs=2, space="PSUM") as psY:
            for qi in (3, 2, 1, 0):
                qs = slice(qi * 512, (qi + 1) * 512)
                nch = 4 * (qi + 1)
                # diagonal (masked) chunks first: their longer pipeline
                # overlaps the later mask-free chunks
                js = list(range(4 * qi, nch)) + list(range(0, 4 * qi))
                for p in range(2):
                    o_ps = []
                    for hl in range(2):
                        o = psO.tile([65, 512], F32, tag="o",
                                     name=f"o{qi}{p}{hl}")
                        o_ps.append(o)
                    for ji, j in enumerate(js):
                        jp = j - 4 * qi
                        st = psB.tile([128, 1024], F32, tag="st",
                                      name=f"st{p}{j}")
                        for hl in range(2):
                            rows = slice(hl * 64, hl * 64 + 64)
                            half = slice(hl * 512, hl * 512 + 512)
                            nc.tensor.matmul(
                                st[:, half], kt_sb[p][rows, j * 128:(j + 1) * 128],
                                qt_sb[p][rows, qs], start=True,
                                stop=(jp < 0))
                            if jp >= 0:
                                # add -240 to masked (k>q) positions on PE
                                nc.tensor.matmul(
                                    st[:, half], ident[:], mask_t[jp][:],
                                    start=False, stop=True)
                        pt = ptp.tile([128, 1024], F32R, tag="pt",
                                      name=f"pt{p}{j}")
                        nc.scalar.activation(pt[:], st[:], EXP, scale=SCALE)
                        for hl in range(2):
                            h = 2 * p + hl
                            off = h * VSTRIDE + j * 65
                            nc.tensor.matmul(
                                o_ps[hl][:], v_sb[:, off:off + 65],
                                pt[:, hl * 512:hl * 512 + 512],
                                start=(ji == 0), stop=(ji == nch - 1))
                    for hl in range(2):
                        rows = slice(hl * 64, hl * 64 + 64)
                        idx = (qi * 2 + p) * 2 + hl
                        ou = phB.tile([64, 512], F32, tag="ou",
                                      name=f"ou{p}{hl}")
                        nc.vector.tensor_copy(ou[:], o_ps[hl][0:64, :])
                        s_sb = small.tile([1, 512], F32, tag="s_sb")
                        nc.vector.tensor_copy(s_sb[:], o_ps[hl][64:65, :])
                        nc.scalar.dma_start(rb_d[idx:idx + 1, :], s_sb[0:1, :])
                        s_pd = small.tile([128, 4], F32, tag="s_pd")
                        nc.scalar.dma_start(
                            s_pd[:, :],
                            rb_d[idx:idx + 1, :].rearrange(
                                "o (p f) -> (o p) f", p=128))
                        r_pd = small.tile([128, 4], F32, tag="r_pd")
                        nc.vector.reciprocal(r_pd[:], s_pd[:])
                        nc.scalar.dma_start(
                            rb2_d[idx:idx + 1, :].rearrange(
                                "o (p f) -> (o p) f", p=128),
                            r_pd[:, :])
                        rb_sb = rbp.tile([64, 512], F32, tag="rb_sb")
                        nc.scalar.dma_start(
                            rb_sb[:, :],
                            rb2_d[idx:idx + 1, :].to_broadcast((64, 512)))
                        nc.vector.tensor_mul(outT[p][rows, qs], ou[:],
                                             rb_sb[:])
                # output projection for this q-tile; y DMAd straight from PSUM
                for qc in range(4 * qi, 4 * qi + 4):
                    qcs = slice(qc * 128, (qc + 1) * 128)
                    for ct in range(2):
                        y_ps = psY.tile([128, 512], F32, tag="y",
                                        name=f"y{qc}{ct}")
                        for p in range(2):
                            nc.tensor.matmul(
                                y_ps[:], outT[p][:, qcs],
                                wout_t[p][:, ct * 512:(ct + 1) * 512],
                                start=(p == 0), stop=(p == 1))
                        y_sb = phB.tile([128, 512], F32, tag="y_sb",
                                        name=f"ysb{qc}{ct}")
                        nc.vector.tensor_copy(y_sb[:], y_ps[:])
                        nc.sync.dma_start(y_d[qcs, ct * 512:(ct + 1) * 512],
                                          y_sb[:])

    nc.compile()
    return nc


_NC = None


def _get_nc():
    global _NC
    if _NC is None:
        _NC = _build_nc()
    return _NC


def _host_shards(x, w_qkv, b_qkv, w_out, b_out, pos_bias):
    x = np.asarray(x, dtype=np.float32)
    w_qkv = np.asarray(w_qkv, dtype=np.float32)
    b_qkv = np.asarray(b_qkv, dtype=np.float32)
    w_out = np.asarray(w_out, dtype=np.float32)
    pos_bias = np.asarray(pos_bias, dtype=np.float32).reshape(HEADS, DH)

    wq, wk, wv = w_qkv[:, :DIM], w_qkv[:, DIM:2 * DIM], w_qkv[:, 2 * DIM:]
    bq, bk, bv = b_qkv[:DIM], b_qkv[DIM:2 * DIM], b_qkv[2 * DIM:]

    jj = np.arange(4)[:, None, None]
    dk = np.arange(128)[None, :, None]
    dq = np.arange(512)[None, None, :]
    masks = np.where(128 * jj + dk <= dq, 0.0, -240.0).astype(np.float32)
    ident = np.eye(128, dtype=np.float32)

    maps = []
    for core in range(NCORES):
        b, g = divmod(core, HPC)
        h0 = HPC * g
        cols = slice(h0 * DH, (h0 + HPC) * DH)          # 256 head dims
        pair_cols = [slice((h0 + 2 * p) * DH, (h0 + 2 * p + 2) * DH)
                     for p in range(2)]
        wqk_c = np.concatenate(
            [wq[:, pair_cols[0]], wq[:, pair_cols[1]],
             wk[:, pair_cols[0]], wk[:, pair_cols[1]]], axis=1)
        qbias = np.stack(
            [bq[pair_cols[p]]
             + pos_bias[h0 + 2 * p:h0 + 2 * p + 2].reshape(-1)
             for p in range(2)], axis=1)
        kbias = np.stack([bk[pair_cols[p]] for p in range(2)], axis=1)
        bvb = np.broadcast_to(bv[cols], (128, HPC * DH))
        wout_c = np.stack([w_out[pair_cols[p], :] for p in range(2)])
        maps.append({
            "xt": np.ascontiguousarray(x[b].T),
            "wqk": np.ascontiguousarray(wqk_c),
            "wv": np.ascontiguousarray(wv[:, cols]),
            "qbias": np.ascontiguousarray(qbias),
            "kbias": np.ascontiguousarray(kbias),
            "bvb": np.ascontiguousarray(bvb),
            "wout": np.ascontiguousarray(wout_c),
            "masks": masks,
            "ident": ident,
        })
    return maps


def kernel(x, w_qkv, b_qkv, w_out, b_out, pos_bias, _trace=False):
    nc = _get_nc()
    in_maps = _host_shards(x, w_qkv, b_qkv, w_out, b_out, pos_bias)
    res = run_bass_kernel_spmd(nc, in_maps, list(range(NCORES)),
                               trace=_trace)
    b_out = np.asarray(b_out, dtype=np.float32)
    y = np.empty((B, T, DIM), dtype=np.float32)
    for b in range(B):
        acc = res.results[b * HPC]["y"].astype(np.float64)
        for g in range(1, HPC):
            acc = acc + res.results[b * HPC + g]["y"]
        y[b] = (acc + b_out).astype(np.float32)
    if _trace:
        kernel._last_results = res
    return y



# revision 11
# speedup vs baseline: 1.0353x; 1.0353x over previous
"""Causal temporal attention kernel for 8 Trainium2 NeuronCores.

Reference computation (per batch b):
    qkv = x @ w_qkv + b_qkv ; split into q,k,v heads [H=16, Dh=64]
    q += pos_bias ; S = q k^T * Dh^-0.5 ; causal softmax ; out = S v
    y = concat_heads(out) @ w_out + b_out

Sharding: batch 2-way x head-group 4-way -> 8 cores. Core c = b*4 + g
computes heads 4g..4g+3 of batch b and returns the partial
y_part = concat(out_heads) @ w_out[rows of its heads]  ([T, DIM], bf16).
Host sums the 4 partials per batch and adds b_out.

v2 layout (same math as v1, restructured schedule):
  * QKV projection is produced per 512-token slice and software-pipelined
    with the attention consumer: round r emits attention for q-tile r-1
    interleaved with projection of slice r, so the ACT engine starts
    exp'ing at ~15us instead of ~107us and the PE never idles on DMA.
  * PSUM budget (8 banks): "proj" tag [128,512]x2 (QKT groups, V chunks
    and y-projection tiles all share it), "st" [128,1024]x2, "o"
    [128,512]x2. The o bank holds the AV numerator+denominator in rows
    0-64 and, after a PE outer-product broadcast of 1/denom (ones[1,64]
    (x) r[1,512]), the per-q reciprocal in rows 64-127 -- the DVE then
    multiplies the two halves straight out of PSUM. This replaces v1's
    DRAM-round-trip partition broadcast (no DMAs on the ACT queue at
    all now).
  * Diagonal 128x128 blocks are masked by a single resident bf16
    triangle matmul; S^T/exp/AV are narrowed to the causally live
    columns of diagonal chunks.
  * Input DMAs issue from the sync queue, xt slices 1-3 and y stores
    from the gpsimd queue; nothing blocks Scalar/Vector.
"""

import sys

sys.path.insert(0, "/opt/trn_rl_repo")

from contextlib import ExitStack

import numpy as np

import concourse.bacc as bacc
import concourse.tile as tile
from concourse import mybir
from concourse.bass_utils import run_bass_kernel_spmd

F32 = mybir.dt.float32
F32R = mybir.dt.float32r
BF16 = mybir.dt.bfloat16
EXP = mybir.ActivationFunctionType.Exp

B, T, DIM = 2, 2048, 1024
HEADS, DH = 16, 64
HPC = 4              # heads per core
NCORES = 8
SCALE = DH ** -0.5
NSL = 4              # 512-token slices / q-tiles
KCH = T // 128       # 16 k-chunks of 128
VSTRIDE = KCH * 65   # per-head stride in v_sb
NARROW_EXP = True


def _merge(a, b):
    """Proportionally interleave two unit generators (lists of thunks)."""
    out = []
    ia = ib = 0
    while ia < len(a) or ib < len(b):
        if ib >= len(b) or (ia < len(a) and ia * (len(b) or 1) <= ib * (len(a) or 1)):
            out.append(a[ia]); ia += 1
        else:
            out.append(b[ib]); ib += 1
    return out


def _build_nc():
    nc = bacc.Bacc("TRN2", target_bir_lowering=False, debug=False,
                   num_devices=NCORES)
    xt_d = nc.dram_tensor("xt", [DIM, T], F32, kind="ExternalInput").ap()
    wqk_d = nc.dram_tensor("wqk", [DIM, 512], F32, kind="ExternalInput").ap()
    wv_d = nc.dram_tensor("wv", [DIM, HPC * DH], F32, kind="ExternalInput").ap()
    qb_d = nc.dram_tensor("qbias", [128, 2], F32, kind="ExternalInput").ap()
    kb_d = nc.dram_tensor("kbias", [128, 2], F32, kind="ExternalInput").ap()
    bvb_d = nc.dram_tensor("bvb", [128, HPC * DH], F32, kind="ExternalInput").ap()
    wout_d = nc.dram_tensor("wout", [2, 128, DIM], F32, kind="ExternalInput").ap()
    mask_d = nc.dram_tensor("masktri", [128, 128], BF16, kind="ExternalInput").ap()
    id_d = nc.dram_tensor("ident", [128, 128], BF16, kind="ExternalInput").ap()
    y_d = nc.dram_tensor("y", [T, DIM], BF16, kind="ExternalOutput").ap()

    with tile.TileContext(nc) as tc, ExitStack() as ctx:
        res = ctx.enter_context(tc.tile_pool(name="res", bufs=1))
        small = ctx.enter_context(tc.tile_pool(name="small", bufs=4))

        # ---- resident input tiles + DMA issue plan ----
        wqk_t, wv_t = [], []
        xt_t = {}
        for c in range(8):
            w = res.tile([128, 512], F32R, tag=f"wqk{c}", name=f"wqk{c}")
            wqk_t.append(w)
            w = res.tile([128, HPC * DH], F32R, tag=f"wv{c}", name=f"wv{c}")
            wv_t.append(w)
            for sl in range(NSL):
                t_ = res.tile([128, 512], F32R, tag=f"xt{c}_{sl}",
                              name=f"xt{c}_{sl}")
                xt_t[(c, sl)] = t_
        qb = res.tile([128, 2], F32, tag="qb")
        kb = res.tile([128, 2], F32, tag="kb")
        bvb = res.tile([128, HPC * DH], F32, tag="bvb")
        maskT = res.tile([128, 128], BF16, tag="maskT")
        ident = res.tile([128, 128], BF16, tag="ident")
        wout_t = []
        for p in range(2):
            w = res.tile([128, DIM], F32R, tag=f"wout{p}", name=f"wout{p}")
            wout_t.append(w)

        # sync queue: weights + slice 0, then small tensors, then wout.
        for c in range(8):
            nc.sync.dma_start(wqk_t[c][:],
                              wqk_d[c * 128:(c + 1) * 128, :].bitcast(F32R))
            nc.sync.dma_start(xt_t[(c, 0)][:],
                              xt_d[c * 128:(c + 1) * 128, 0:512].bitcast(F32R))
            nc.sync.dma_start(wv_t[c][:],
                              wv_d[c * 128:(c + 1) * 128, :].bitcast(F32R))
        nc.sync.dma_start(qb[:], qb_d[:, :])
        nc.sync.dma_start(kb[:], kb_d[:, :])
        nc.sync.dma_start(bvb[:], bvb_d[:, :])
        nc.sync.dma_start(maskT[:], mask_d[:, :])
        nc.sync.dma_start(ident[:], id_d[:, :])
        for p in range(2):
            nc.sync.dma_start(wout_t[p][:], wout_d[p].bitcast(F32R))

        # ---- persistent compute tiles ----
        qt_sb, kt_sb, outT = [], [], []
        for p in range(2):
            qt_sb.append(res.tile([128, T], F32R, tag=f"qt{p}", name=f"qt{p}"))
            kt_sb.append(res.tile([128, T], F32R, tag=f"kt{p}", name=f"kt{p}"))
            outT.append(res.tile([128, T], F32R, tag=f"outT{p}", name=f"outT{p}"))
        v_sb = res.tile([128, HPC * VSTRIDE], F32R, tag="v_sb")

        ones_f = small.tile([128, 512], F32, tag="ones_f")
        nc.any.memset(ones_f[:], 1.0)
        warm = res.tile([1, 512], F32R, tag="warm")
        nc.vector.tensor_copy(warm[:], ones_f[0:1, :])
        ones64 = res.tile([1, 64], F32R, tag="ones64")
        nc.vector.tensor_copy(ones64[:], ones_f[0:1, 0:64])

        # v_sb denominator-ones columns (written once, never touched again)
        for h in range(HPC):
            for m in range(KCH):
                off = h * VSTRIDE + m * 65 + 64
                nc.vector.tensor_copy(v_sb[:, off:off + 1], ones_f[:, 0:1])
        # gpsimd queue: late xt slices.
        for sl in range(1, NSL):
            for c in range(8):
                nc.gpsimd.dma_start(
                    xt_t[(c, sl)][:],
                    xt_d[c * 128:(c + 1) * 128,
                         sl * 512:(sl + 1) * 512].bitcast(F32R))

        # ---- PE warm-up burst while the first DMAs stream in ----
        with tc.tile_pool(name="psW", bufs=2, space="PSUM") as psW:
            for i in range(28):
                wp = psW.tile([64, 512], F32, tag="warm_ps", name=f"warm{i}")
                nc.tensor.matmul(wp[:], ones64[:], warm[:], start=True,
                                 stop=True)

        # ---- main pools: exactly 8 PSUM banks ----
        psP = ctx.enter_context(tc.tile_pool(name="psP", bufs=2, space="PSUM"))
        psS = ctx.enter_context(tc.tile_pool(name="psS", bufs=2, space="PSUM"))
        psO = ctx.enter_context(tc.tile_pool(name="psO", bufs=2, space="PSUM"))
        ptp = ctx.enter_context(tc.tile_pool(name="ptp", bufs=5))
        ysp = ctx.enter_context(tc.tile_pool(name="ysp", bufs=3))
        rsp = ctx.enter_context(tc.tile_pool(name="rsp", bufs=2))

        def proj_units(sl):
            """QKV projection for token slice sl -> list of thunks."""
            units = []
            # QT/KT: groups g = qk*2+p; two [128,512] banks at a time.
            for blk in range(2):
                state = {}

                def open_blk(state=state, blk=blk, sl=sl):
                    state["ps"] = [
                        psP.tile([128, 512], F32, tag="proj",
                                 name=f"qkps{sl}{blk}{gi}")
                        for gi in range(2)]

                for cp in range(4):
                    def run(cp=cp, blk=blk, sl=sl, state=state, ob=open_blk):
                        if cp == 0:
                            ob()
                        for c in (2 * cp, 2 * cp + 1):
                            for gi in range(2):
                                g = blk * 2 + gi
                                qk, p = divmod(g, 2)
                                col = (qk * 2 + p) * 128
                                nc.tensor.matmul(
                                    state["ps"][gi][:],
                                    wqk_t[c][:, col:col + 128],
                                    xt_t[(c, sl)][:],
                                    start=(c == 0), stop=(c == 7))
                    units.append(run)

                def evac(blk=blk, sl=sl, state=state):
                    for gi in range(2):
                        g = blk * 2 + gi
                        qk, p = divmod(g, 2)
                        dst = (qt_sb if qk == 0 else kt_sb)[p]
                        bias = (qb if qk == 0 else kb)[:, p:p + 1]
                        nc.vector.tensor_add(
                            dst[:, sl * 512:(sl + 1) * 512], state["ps"][gi][:],
                            bias.to_broadcast((128, 512)))
                units.append(evac)
            # V: m-chunks, two [128,256] outputs per block in proj banks.
            for blk in range(2):
                state = {}

                def open_blk(state=state, blk=blk, sl=sl):
                    state["ps"] = [
                        psP.tile([128, 512], F32, tag="proj",
                                 name=f"vps{sl}{blk}{mi}")
                        for mi in range(2)]

                for cp in range(4):
                    def run(cp=cp, blk=blk, sl=sl, state=state, ob=open_blk):
                        if cp == 0:
                            ob()
                        for c in (2 * cp, 2 * cp + 1):
                            for mi in range(2):
                                ml = blk * 2 + mi
                                nc.tensor.matmul(
                                    state["ps"][mi][:, 0:HPC * DH],
                                    xt_t[(c, sl)][:, ml * 128:(ml + 1) * 128],
                                    wv_t[c][:],
                                    start=(c == 0), stop=(c == 7))
                    units.append(run)

                def evac(blk=blk, sl=sl, state=state):
                    for mi in range(2):
                        m = sl * 4 + blk * 2 + mi
                        for h in range(HPC):
                            off = h * VSTRIDE + m * 65
                            nc.vector.tensor_add(
                                v_sb[:, off:off + 64],
                                state["ps"][mi][:, h * DH:(h + 1) * DH],
                                bvb[:, h * DH:(h + 1) * DH])
                units.append(evac)
            return units

        def attn_units(qi):
            """Attention + output projection for q-tile qi."""
            units = []
            qs = slice(qi * 512, (qi + 1) * 512)
            js = list(range(4 * qi, 4 * qi + 4)) + list(range(0, 4 * qi))
            for p in range(2):
                state = {"prev": None, "o": None}

                def chunk(ji, j, p=p, qi=qi, state=state, js=js):
                    if ji == 0:
                        state["o"] = [
                            psO.tile([128, 512], F32, tag="o",
                                     name=f"o{qi}{p}{hl}")
                            for hl in range(2)]
                    if j is not None:
                        jl = j - 4 * qi
                        w0 = 128 * jl if jl >= 0 else 0
                        st = psS.tile([128, 1024], F32, tag="st",
                                      name=f"st{qi}{p}{j}")
                        for hl in range(2):
                            rows = slice(hl * 64, hl * 64 + 64)
                            nc.tensor.matmul(
                                st[:, hl * 512 + w0:(hl + 1) * 512],
                                kt_sb[p][rows, j * 128:(j + 1) * 128],
                                qt_sb[p][rows, qi * 512 + w0:(qi + 1) * 512],
                                start=True, stop=(jl < 0))
                            if jl >= 0:
                                nc.tensor.matmul(
                                    st[:, hl * 512 + w0:hl * 512 + w0 + 128],
                                    ident[:], maskT[:],
                                    start=False, stop=True)
                        pt = ptp.tile([128, 1024], F32R, tag="pt",
                                      name=f"pt{qi}{p}{j}")
                        if w0 and NARROW_EXP:
                            stv = st.rearrange("a (h q) -> a h q", h=2)
                            ptv = pt.rearrange("a (h q) -> a h q", h=2)
                            nc.scalar.activation(ptv[:, :, w0:512],
                                                 stv[:, :, w0:512], EXP,
                                                 scale=SCALE)
                        else:
                            nc.scalar.activation(pt[:], st[:], EXP,
                                                 scale=SCALE)
                        state["cur"] = (j, pt, w0)
                    else:
                        state["cur"] = None
                    if state["prev"] is not None:
                        jp, ptq, w0p = state["prev"]
                        for hl in range(2):
                            h = 2 * p + hl
                            off = h * VSTRIDE + jp * 65
                            nc.tensor.matmul(
                                state["o"][hl][0:65, w0p:512],
                                v_sb[:, off:off + 65],
                                ptq[:, hl * 512 + w0p:(hl + 1) * 512],
                                start=(ji == 1), stop=(ji == len(js)))
                    state["prev"] = state["cur"]

                for ji, j in enumerate(js + [None]):
                    units.append(lambda ji=ji, j=j, fn=chunk: fn(ji, j))

                def norm(p=p, qi=qi, state=state):
                    for hl in range(2):
                        rows = slice(hl * 64, hl * 64 + 64)
                        o = state["o"][hl]
                        r = rsp.tile([1, 512], F32R, tag="r",
                                     name=f"r{qi}{p}{hl}")
                        with nc.allow_low_precision(
                                reason="f32r out is bit-identical to f32"):
                            nc.vector.reciprocal(r[:], o[64:65, :])
                        bc = psP.tile([128, 512], F32, tag="proj",
                                      name=f"bc{qi}{p}{hl}")
                        nc.tensor.matmul(bc[0:64, :], ones64[:], r[:],
                                         start=True, stop=True)
                        rb = rsp.tile([64, 512], F32R, tag="rb",
                                      name=f"rb{qi}{p}{hl}")
                        nc.vector.tensor_copy(rb[:], bc[0:64, :])
                        nc.vector.tensor_mul(
                            outT[p][rows, qi * 512:(qi + 1) * 512],
                            o[0:64, :], rb[:])
                units.append(norm)
            for qc in range(4 * qi, 4 * qi + 4):
                def yproj(qc=qc):
                    qcs = slice(qc * 128, (qc + 1) * 128)
                    for ct in range(2):
                        y_ps = psP.tile([128, 512], F32, tag="proj",
                                        name=f"y{qc}{ct}")
                        for p in range(2):
                            nc.tensor.matmul(
                                y_ps[:], outT[p][:, qcs],
                                wout_t[p][:, ct * 512:(ct + 1) * 512],
                                start=(p == 0), stop=(p == 1))
                        y_sb = ysp.tile([128, 512], BF16, tag="ysb",
                                        name=f"ysb{qc}{ct}")
                        nc.vector.tensor_copy(y_sb[:], y_ps[:])
                        nc.gpsimd.dma_start(y_d[qcs, ct * 512:(ct + 1) * 512],
                                            y_sb[:])
                units.append(yproj)
            return units

        # ---- emit rounds ----
        rounds = [proj_units(0)]
        for qi in range(NSL):
            a = attn_units(qi)
            b = proj_units(qi + 1) if qi + 1 < NSL else []
            rounds.append(_merge(a, b))
        for rr in rounds:
            for u in rr:
                u()

    nc.compile()
    return nc


_NC = None


def _get_nc():
    global _NC
    if _NC is None:
        _NC = _build_nc()
    return _NC


def _host_shards(x, w_qkv, b_qkv, w_out, b_out, pos_bias):
    import ml_dtypes
    x = np.asarray(x, dtype=np.float32)
    w_qkv = np.asarray(w_qkv, dtype=np.float32)
    b_qkv = np.asarray(b_qkv, dtype=np.float32)
    w_out = np.asarray(w_out, dtype=np.float32)
    pos_bias = np.asarray(pos_bias, dtype=np.float32).reshape(HEADS, DH)

    wq, wk, wv = w_qkv[:, :DIM], w_qkv[:, DIM:2 * DIM], w_qkv[:, 2 * DIM:]
    bq, bk, bv = b_qkv[:DIM], b_qkv[DIM:2 * DIM], b_qkv[2 * DIM:]

    dk = np.arange(128)[:, None]
    dq = np.arange(128)[None, :]
    masktri = np.where(dk <= dq, 0.0, -240.0).astype(ml_dtypes.bfloat16)
    ident = np.eye(128, dtype=ml_dtypes.bfloat16)

    maps = []
    for core in range(NCORES):
        b, g = divmod(core, HPC)
        h0 = HPC * g
        cols = slice(h0 * DH, (h0 + HPC) * DH)          # 256 head dims
        pair_cols = [slice((h0 + 2 * p) * DH, (h0 + 2 * p + 2) * DH)
                     for p in range(2)]
        wqk_c = np.concatenate(
            [wq[:, pair_cols[0]], wq[:, pair_cols[1]],
             wk[:, pair_cols[0]], wk[:, pair_cols[1]]], axis=1)
        qbias = np.stack(
            [bq[pair_cols[p]]
             + pos_bias[h0 + 2 * p:h0 + 2 * p + 2].reshape(-1)
             for p in range(2)], axis=1)
        kbias = np.stack([bk[pair_cols[p]] for p in range(2)], axis=1)
        bvb = np.broadcast_to(bv[cols], (128, HPC * DH))
        wout_c = np.stack([w_out[pair_cols[p], :] for p in range(2)])
        maps.append({
            "xt": np.ascontiguousarray(x[b].T),
            "wqk": np.ascontiguousarray(wqk_c),
            "wv": np.ascontiguousarray(wv[:, cols]),
            "qbias": np.ascontiguousarray(qbias),
            "kbias": np.ascontiguousarray(kbias),
            "bvb": np.ascontiguousarray(bvb),
            "wout": np.ascontiguousarray(wout_c),
            "masktri": masktri,
            "ident": ident,
        })
    return maps


def kernel(x, w_qkv, b_qkv, w_out, b_out, pos_bias, _trace=False):
    nc = _get_nc()
    in_maps = _host_shards(x, w_qkv, b_qkv, w_out, b_out, pos_bias)
    res = run_bass_kernel_spmd(nc, in_maps, list(range(NCORES)),
                               trace=_trace)
    b_out = np.asarray(b_out, dtype=np.float32)
    y = np.empty((B, T, DIM), dtype=np.float32)
    for b in range(B):
        acc = res.results[b * HPC]["y"].astype(np.float32)
        for g in range(1, HPC):
            acc = acc + res.results[b * HPC + g]["y"].astype(np.float32)
        y[b] = acc + b_out
    if _trace:
        kernel._last_results = res
    return y


# revision 12
# speedup vs baseline: 1.2080x; 1.1669x over previous
"""Causal temporal attention kernel for 8 Trainium2 NeuronCores.

Reference computation (per batch b):
    qkv = x @ w_qkv + b_qkv ; split into q,k,v heads [H=16, Dh=64]
    q += pos_bias ; S = q k^T * Dh^-0.5 ; causal softmax ; out = S v
    y = concat_heads(out) @ w_out + b_out

Sharding: batch 2-way x head-group 4-way -> 8 cores. Core c = b*4 + g
computes heads 4g..4g+3 of batch b and returns the partial
y_part = concat(out_heads) @ w_out[rows of its heads]  ([T, DIM], bf16).
Host sums the 4 partials per batch and adds b_out.

v2 layout (same math as v1, restructured schedule):
  * QKV projection is produced per 512-token slice and software-pipelined
    with the attention consumer: round r emits attention for q-tile r-1
    interleaved with projection of slice r, so the ACT engine starts
    exp'ing at ~15us instead of ~107us and the PE never idles on DMA.
  * PSUM budget (8 banks): "proj" tag [128,512]x2 (QKT groups, V chunks
    and y-projection tiles all share it), "st" [128,1024]x2, "o"
    [128,512]x2. The o bank holds the AV numerator+denominator in rows
    0-64 and, after a PE outer-product broadcast of 1/denom (ones[1,64]
    (x) r[1,512]), the per-q reciprocal in rows 64-127 -- the DVE then
    multiplies the two halves straight out of PSUM. This replaces v1's
    DRAM-round-trip partition broadcast (no DMAs on the ACT queue at
    all now).
  * Diagonal 128x128 blocks are masked by a single resident bf16
    triangle matmul; S^T/exp/AV are narrowed to the causally live
    columns of diagonal chunks.
  * Input DMAs issue from the sync queue, xt slices 1-3 and y stores
    from the gpsimd queue; nothing blocks Scalar/Vector.
"""

import sys

sys.path.insert(0, "/opt/trn_rl_repo")

from contextlib import ExitStack

import numpy as np

import concourse.bacc as bacc
import concourse.tile as tile
from concourse import mybir
from concourse.bass_utils import run_bass_kernel_spmd

F32 = mybir.dt.float32
F32R = mybir.dt.float32r
BF16 = mybir.dt.bfloat16
EXP = mybir.ActivationFunctionType.Exp

B, T, DIM = 2, 2048, 1024
HEADS, DH = 16, 64
HPC = 4              # heads per core
NCORES = 8
SCALE = DH ** -0.5
NSL = 4              # 512-token slices / q-tiles
KCH = T // 128       # 16 k-chunks of 128
VSTRIDE = KCH * 65   # per-head stride in v_sb
NARROW_EXP = True


def _merge(a, b):
    """Proportionally interleave two unit generators (lists of thunks)."""
    out = []
    ia = ib = 0
    while ia < len(a) or ib < len(b):
        if ib >= len(b) or (ia < len(a) and ia * (len(b) or 1) <= ib * (len(a) or 1)):
            out.append(a[ia]); ia += 1
        else:
            out.append(b[ib]); ib += 1
    return out


def _build_nc():
    nc = bacc.Bacc("TRN2", target_bir_lowering=False, debug=False,
                   num_devices=NCORES)
    xt_d = nc.dram_tensor("xt", [DIM, T], BF16, kind="ExternalInput").ap()
    wqk_d = nc.dram_tensor("wqk", [DIM, 512], BF16, kind="ExternalInput").ap()
    wv_d = nc.dram_tensor("wv", [DIM, HPC * DH], BF16, kind="ExternalInput").ap()
    qb_d = nc.dram_tensor("qbias", [128, 2], F32, kind="ExternalInput").ap()
    kb_d = nc.dram_tensor("kbias", [128, 2], F32, kind="ExternalInput").ap()
    bvb_d = nc.dram_tensor("bvb", [128, HPC * DH], F32, kind="ExternalInput").ap()
    wout_d = nc.dram_tensor("wout", [2, 128, DIM], BF16, kind="ExternalInput").ap()
    mask_d = nc.dram_tensor("masktri", [128, 128], BF16, kind="ExternalInput").ap()
    y_d = nc.dram_tensor("y", [T, DIM], BF16, kind="ExternalOutput").ap()
    rb_d = nc.dram_tensor("rbscratch", [2 * NSL * 2, 512], F32).ap()
    rb2_d = nc.dram_tensor("rbscratch2", [2 * NSL * 2, 512], F32).ap()

    with tile.TileContext(nc) as tc, ExitStack() as ctx:
        res = ctx.enter_context(tc.tile_pool(name="res", bufs=1))
        small = ctx.enter_context(tc.tile_pool(name="small", bufs=4))

        # ---- resident input tiles + DMA issue plan ----
        wqk_t, wv_t = [], []
        xt_t = {}
        for c in range(8):
            w = res.tile([128, 512], BF16, tag=f"wqk{c}", name=f"wqk{c}")
            wqk_t.append(w)
            w = res.tile([128, HPC * DH], BF16, tag=f"wv{c}", name=f"wv{c}")
            wv_t.append(w)
            for sl in range(NSL):
                t_ = res.tile([128, 512], BF16, tag=f"xt{c}_{sl}",
                              name=f"xt{c}_{sl}")
                xt_t[(c, sl)] = t_
        qb = res.tile([128, 2], F32, tag="qb")
        kb = res.tile([128, 2], F32, tag="kb")
        bvb = res.tile([128, HPC * DH], F32, tag="bvb")
        maskT = res.tile([128, 128], BF16, tag="maskT")
        wout_t = []
        for p in range(2):
            w = res.tile([128, DIM], BF16, tag=f"wout{p}", name=f"wout{p}")
            wout_t.append(w)

        # sync queue: weights + slice 0, then small tensors, then wout.
        for c in range(8):
            nc.sync.dma_start(wqk_t[c][:],
                              wqk_d[c * 128:(c + 1) * 128, :])
            nc.sync.dma_start(xt_t[(c, 0)][:],
                              xt_d[c * 128:(c + 1) * 128, 0:512])
            nc.sync.dma_start(wv_t[c][:],
                              wv_d[c * 128:(c + 1) * 128, :])
        nc.sync.dma_start(qb[:], qb_d[:, :])
        nc.sync.dma_start(kb[:], kb_d[:, :])
        nc.sync.dma_start(bvb[:], bvb_d[:, :])
        nc.sync.dma_start(maskT[:], mask_d[:, :])
        for p in range(2):
            nc.sync.dma_start(wout_t[p][:], wout_d[p])

        # ---- persistent compute tiles ----
        qt_sb, kt_sb, outT = [], [], []
        for p in range(2):
            qt_sb.append(res.tile([128, T], BF16, tag=f"qt{p}", name=f"qt{p}"))
            kt_sb.append(res.tile([128, T], BF16, tag=f"kt{p}", name=f"kt{p}"))
            outT.append(res.tile([128, T], BF16, tag=f"outT{p}", name=f"outT{p}"))
        v_sb = res.tile([128, HPC * VSTRIDE], BF16, tag="v_sb")

        ones_f = small.tile([128, 512], F32, tag="ones_f")
        nc.any.memset(ones_f[:], 1.0)
        warm = res.tile([1, 512], F32R, tag="warm")
        nc.vector.tensor_copy(warm[:], ones_f[0:1, :])
        ones64 = res.tile([1, 64], F32R, tag="ones64")
        nc.vector.tensor_copy(ones64[:], ones_f[0:1, 0:64])

        # v_sb denominator-ones columns (written once, never touched again)
        for h in range(HPC):
            for m in range(KCH):
                off = h * VSTRIDE + m * 65 + 64
                nc.vector.tensor_copy(v_sb[:, off:off + 1], ones_f[:, 0:1])
        # gpsimd queue: late xt slices.
        for sl in range(1, NSL):
            for c in range(8):
                nc.gpsimd.dma_start(
                    xt_t[(c, sl)][:],
                    xt_d[c * 128:(c + 1) * 128,
                         sl * 512:(sl + 1) * 512])

        # ---- PE warm-up burst while the first DMAs stream in ----
        with tc.tile_pool(name="psW", bufs=2, space="PSUM") as psW:
            for i in range(28):
                wp = psW.tile([64, 512], F32, tag="warm_ps", name=f"warm{i}")
                nc.tensor.matmul(wp[:], ones64[:], warm[:], start=True,
                                 stop=True)

        # ---- main pools: exactly 8 PSUM banks ----
        psP = ctx.enter_context(tc.tile_pool(name="psP", bufs=2, space="PSUM"))
        psS = ctx.enter_context(tc.tile_pool(name="psS", bufs=2, space="PSUM"))
        psO = ctx.enter_context(tc.tile_pool(name="psO", bufs=2, space="PSUM"))
        ptp = ctx.enter_context(tc.tile_pool(name="ptp", bufs=8))
        ysp = ctx.enter_context(tc.tile_pool(name="ysp", bufs=3))
        rbp = ctx.enter_context(tc.tile_pool(name="rbp", bufs=3))
        nsm = ctx.enter_context(tc.tile_pool(name="nsm", bufs=6))

        def proj_units(sl):
            """QKV projection for token slice sl -> list of thunks."""
            units = []
            # QT/KT: groups g = qk*2+p; two [128,512] banks at a time.
            for blk in range(2):
                state = {}

                def open_blk(state=state, blk=blk, sl=sl):
                    state["ps"] = [
                        psP.tile([128, 512], F32, tag="proj",
                                 name=f"qkps{sl}{blk}{gi}")
                        for gi in range(2)]

                for cp in range(4):
                    def run(cp=cp, blk=blk, sl=sl, state=state, ob=open_blk):
                        if cp == 0:
                            ob()
                        for c in (2 * cp, 2 * cp + 1):
                            for gi in range(2):
                                g = blk * 2 + gi
                                qk, p = divmod(g, 2)
                                col = (qk * 2 + p) * 128
                                nc.tensor.matmul(
                                    state["ps"][gi][:],
                                    wqk_t[c][:, col:col + 128],
                                    xt_t[(c, sl)][:],
                                    start=(c == 0), stop=(c == 7))
                    units.append(run)

                def evac(blk=blk, sl=sl, state=state):
                    for gi in range(2):
                        g = blk * 2 + gi
                        qk, p = divmod(g, 2)
                        dst = (qt_sb if qk == 0 else kt_sb)[p]
                        bias = (qb if qk == 0 else kb)[:, p:p + 1]
                        nc.vector.tensor_add(
                            dst[:, sl * 512:(sl + 1) * 512], state["ps"][gi][:],
                            bias.to_broadcast((128, 512)))
                units.append(evac)
            # V: m-chunks, two [128,256] outputs per block in proj banks.
            for blk in range(2):
                state = {}

                def open_blk(state=state, blk=blk, sl=sl):
                    state["ps"] = [
                        psP.tile([128, 512], F32, tag="proj",
                                 name=f"vps{sl}{blk}{mi}")
                        for mi in range(2)]

                for cp in range(4):
                    def run(cp=cp, blk=blk, sl=sl, state=state, ob=open_blk):
                        if cp == 0:
                            ob()
                        for c in (2 * cp, 2 * cp + 1):
                            for mi in range(2):
                                ml = blk * 2 + mi
                                nc.tensor.matmul(
                                    state["ps"][mi][:, 0:HPC * DH],
                                    xt_t[(c, sl)][:, ml * 128:(ml + 1) * 128],
                                    wv_t[c][:],
                                    start=(c == 0), stop=(c == 7))
                    units.append(run)

                def evac(blk=blk, sl=sl, state=state):
                    for mi in range(2):
                        m = sl * 4 + blk * 2 + mi
                        for h in range(HPC):
                            off = h * VSTRIDE + m * 65
                            nc.vector.tensor_add(
                                v_sb[:, off:off + 64],
                                state["ps"][mi][:, h * DH:(h + 1) * DH],
                                bvb[:, h * DH:(h + 1) * DH])
                units.append(evac)
            return units

        def attn_units(qi):
            """Attention + output projection for q-tile qi."""
            units = []
            qs = slice(qi * 512, (qi + 1) * 512)
            js = list(range(4 * qi, 4 * qi + 4)) + list(range(0, 4 * qi))
            for p in range(2):
                state = {"prev": None, "o": None}

                def chunk(ji, j, p=p, qi=qi, state=state, js=js):
                    if ji == 0:
                        state["o"] = [
                            psO.tile([128, 512], F32, tag="o",
                                     name=f"o{qi}{p}{hl}")
                            for hl in range(2)]
                    if j is not None:
                        jl = j - 4 * qi
                        w0 = 128 * jl if jl >= 0 else 0
                        st = psS.tile([128, 1024], F32, tag="st",
                                      name=f"st{qi}{p}{j}")
                        for hl in range(2):
                            rows = slice(hl * 64, hl * 64 + 64)
                            nc.tensor.matmul(
                                st[:, hl * 512 + w0:(hl + 1) * 512],
                                kt_sb[p][rows, j * 128:(j + 1) * 128],
                                qt_sb[p][rows, qi * 512 + w0:(qi + 1) * 512],
                                start=True, stop=True)
                            if jl >= 0:
                                nc.vector.tensor_add(
                                    st[:, hl * 512 + w0:hl * 512 + w0 + 128],
                                    st[:, hl * 512 + w0:hl * 512 + w0 + 128],
                                    maskT[:])
                        pt = ptp.tile([128, 1024], BF16, tag="pt",
                                      name=f"pt{qi}{p}{j}")
                        if w0 and NARROW_EXP:
                            stv = st.rearrange("a (h q) -> a h q", h=2)
                            ptv = pt.rearrange("a (h q) -> a h q", h=2)
                            nc.scalar.activation(ptv[:, :, w0:512],
                                                 stv[:, :, w0:512], EXP,
                                                 scale=SCALE)
                        else:
                            nc.scalar.activation(pt[:], st[:], EXP,
                                                 scale=SCALE)
                        state["cur"] = (j, pt, w0)
                    else:
                        state["cur"] = None
                    if state["prev"] is not None:
                        jp, ptq, w0p = state["prev"]
                        for hl in range(2):
                            h = 2 * p + hl
                            off = h * VSTRIDE + jp * 65
                            nc.tensor.matmul(
                                state["o"][hl][0:65, w0p:512],
                                v_sb[:, off:off + 65],
                                ptq[:, hl * 512 + w0p:(hl + 1) * 512],
                                start=(ji == 1), stop=(ji == len(js)))
                    state["prev"] = state["cur"]

                for ji, j in enumerate(js + [None]):
                    units.append(lambda ji=ji, j=j, fn=chunk: fn(ji, j))

                def norm(p=p, qi=qi, state=state):
                    for hl in range(2):
                        rows = slice(hl * 64, hl * 64 + 64)
                        o = state["o"][hl]
                        idx = (qi * 2 + p) * 2 + hl
                        s_sb = nsm.tile([1, 512], F32, tag="s_sb")
                        nc.vector.tensor_copy(s_sb[:], o[64:65, :])
                        nc.gpsimd.dma_start(rb_d[idx:idx + 1, :], s_sb[0:1, :])
                        s_pd = nsm.tile([128, 4], F32, tag="s_pd")
                        nc.gpsimd.dma_start(
                            s_pd[:, :],
                            rb_d[idx:idx + 1, :].rearrange(
                                "o (p f) -> (o p) f", p=128))
                        r_pd = nsm.tile([128, 4], F32, tag="r_pd")
                        nc.vector.reciprocal(r_pd[:], s_pd[:])
                        nc.gpsimd.dma_start(
                            rb2_d[idx:idx + 1, :].rearrange(
                                "o (p f) -> (o p) f", p=128),
                            r_pd[:, :])
                        rb = rbp.tile([64, 512], F32, tag="rb",
                                      name=f"rb{qi}{p}{hl}")
                        nc.gpsimd.dma_start(
                            rb[:, :],
                            rb2_d[idx:idx + 1, :].to_broadcast((64, 512)))
                        nc.vector.tensor_mul(
                            outT[p][rows, qi * 512:(qi + 1) * 512],
                            o[0:64, :], rb[:])
                units.append(norm)
            for qc in range(4 * qi, 4 * qi + 4):
                def yproj(qc=qc):
                    qcs = slice(qc * 128, (qc + 1) * 128)
                    for ct in range(2):
                        y_ps = psP.tile([128, 512], F32, tag="proj",
                                        name=f"y{qc}{ct}")
                        for p in range(2):
                            nc.tensor.matmul(
                                y_ps[:], outT[p][:, qcs],
                                wout_t[p][:, ct * 512:(ct + 1) * 512],
                                start=(p == 0), stop=(p == 1))
                        y_sb = ysp.tile([128, 512], BF16, tag="ysb",
                                        name=f"ysb{qc}{ct}")
                        nc.vector.tensor_copy(y_sb[:], y_ps[:])
                        nc.gpsimd.dma_start(y_d[qcs, ct * 512:(ct + 1) * 512],
                                            y_sb[:])
                units.append(yproj)
            return units

        # ---- emit rounds ----
        rounds = [proj_units(0)]
        for qi in range(NSL):
            a = attn_units(qi)
            b = proj_units(qi + 1) if qi + 1 < NSL else []
            rounds.append(_merge(a, b))
        for rr in rounds:
            for u in rr:
                u()

    nc.compile()
    return nc


_NC = None


def _get_nc():
    global _NC
    if _NC is None:
        _NC = _build_nc()
    return _NC


def _host_shards(x, w_qkv, b_qkv, w_out, b_out, pos_bias):
    import ml_dtypes
    x = np.asarray(x, dtype=np.float32)
    w_qkv = np.asarray(w_qkv, dtype=np.float32)
    b_qkv = np.asarray(b_qkv, dtype=np.float32)
    w_out = np.asarray(w_out, dtype=np.float32)
    pos_bias = np.asarray(pos_bias, dtype=np.float32).reshape(HEADS, DH)

    wq, wk, wv = w_qkv[:, :DIM], w_qkv[:, DIM:2 * DIM], w_qkv[:, 2 * DIM:]
    bq, bk, bv = b_qkv[:DIM], b_qkv[DIM:2 * DIM], b_qkv[2 * DIM:]

    dk = np.arange(128)[:, None]
    dq = np.arange(128)[None, :]
    masktri = np.where(dk <= dq, 0.0, -240.0).astype(ml_dtypes.bfloat16)

    maps = []
    for core in range(NCORES):
        b, g = divmod(core, HPC)
        h0 = HPC * g
        cols = slice(h0 * DH, (h0 + HPC) * DH)          # 256 head dims
        pair_cols = [slice((h0 + 2 * p) * DH, (h0 + 2 * p + 2) * DH)
                     for p in range(2)]
        wqk_c = np.concatenate(
            [wq[:, pair_cols[0]], wq[:, pair_cols[1]],
             wk[:, pair_cols[0]], wk[:, pair_cols[1]]], axis=1)
        qbias = np.stack(
            [bq[pair_cols[p]]
             + pos_bias[h0 + 2 * p:h0 + 2 * p + 2].reshape(-1)
             for p in range(2)], axis=1)
        kbias = np.stack([bk[pair_cols[p]] for p in range(2)], axis=1)
        bvb = np.broadcast_to(bv[cols], (128, HPC * DH))
        wout_c = np.stack([w_out[pair_cols[p], :] for p in range(2)])
        maps.append({
            "xt": np.ascontiguousarray(x[b].T).astype(ml_dtypes.bfloat16),
            "wqk": np.ascontiguousarray(wqk_c).astype(ml_dtypes.bfloat16),
            "wv": np.ascontiguousarray(wv[:, cols]).astype(ml_dtypes.bfloat16),
            "qbias": np.ascontiguousarray(qbias),
            "kbias": np.ascontiguousarray(kbias),
            "bvb": np.ascontiguousarray(bvb),
            "wout": np.ascontiguousarray(wout_c).astype(ml_dtypes.bfloat16),
            "masktri": masktri,
        })
    return maps


def kernel(x, w_qkv, b_qkv, w_out, b_out, pos_bias, _trace=False):
    nc = _get_nc()
    in_maps = _host_shards(x, w_qkv, b_qkv, w_out, b_out, pos_bias)
    res = run_bass_kernel_spmd(nc, in_maps, list(range(NCORES)),
                               trace=_trace)
    b_out = np.asarray(b_out, dtype=np.float32)
    y = np.empty((B, T, DIM), dtype=np.float32)
    for b in range(B):
        acc = res.results[b * HPC]["y"].astype(np.float32)
        for g in range(1, HPC):
            acc = acc + res.results[b * HPC + g]["y"].astype(np.float32)
        y[b] = acc + b_out
    if _trace:
        kernel._last_results = res
    return y
